# revision 17
# baseline (speedup 1.0000x reference)
"""KAN layer kernel for TRN2, 8-core SPMD.

Math: out[b,o] = sum_{i,k} relu(x[b,i]*w1[o,i,k] + b1[o,i,k]) * w2[o,i,k] / 32 + b2[o]
With b1 == 0 (guaranteed by the generator) the relu factorizes via
relu(z) = (z + |z|)/2 and |x*w1| = |x|*|w1|:
    S[i,o] = sum_k w1*w2        T[i,o] = sum_k |w1|*w2
    out = (x @ S + |x| @ T) * (1/64) + b2
Two bf16 matmuls plus elementwise prep spread across DVE (products,
k-sums), ACT (|.|, epilogue), GpSimd (one product leg). The output store
is a pair of per-chunk prepared SWDGE scatters (identity indices), each
fired by trigger_dma as its epilogue completes — this skips the HWDGE +
DGE-delay latency of a normal DMA; the scatters ADD into the
zero-initialized output buffer, which equals a store.

Sharding: 4 batch groups x 2 dout groups (core = bi*2 + oj).
Wire format is bf16 (host casts; all arithmetic on device; f32 psum
accumulation); output returns in bf16 and is upcast on host.
"""

import numpy as np

B, DIN, DOUT, K = 2048, 256, 256, 4
N_CORES = 8
BG, OG = 4, 2                      # batch groups x dout groups
BS, OS = B // BG, DOUT // OG       # 512 batch rows, 128 dout cols per core
SCALE = 1.0 / np.sqrt(((DOUT + DIN) / 2) * K)   # 1/32
NT = DIN // 128                    # i-tiles (2)
KO = K * OS                        # 512 cols per weight tensor slab
NIDX = 8                           # identity scatter idxs: 8 int16 cols
WCOLS = 2 + NT * 2 * KO + NIDX     # b2 bits + w1/w2 slabs + idxs
NCH = 2                            # batch chunks
CB = BS // NCH                     # 256 batch cols per chunk

_CACHE = {}


def _build_nc():
    if "nc" in _CACHE:
        return _CACHE["nc"]
    import concourse.bacc as bacc
    import concourse.tile as tile
    from concourse import mybir

    f32 = mybir.dt.float32
    bf16 = mybir.dt.bfloat16
    i16 = mybir.dt.int16
    AF = mybir.ActivationFunctionType
    OP = mybir.AluOpType

    nc = bacc.Bacc("TRN2", target_bir_lowering=False, debug=False,
                   num_devices=N_CORES)
    wb = nc.dram_tensor("wb", [128, WCOLS], bf16, kind="ExternalInput")
    xb = nc.dram_tensor("xb", [128, NT, BS], bf16, kind="ExternalInput")
    outb = nc.dram_tensor("outb", [128, BS], bf16, kind="ExternalOutput")

    W0E = 2 + 2 * KO               # end of [b2 | w1t0 | w2t0]

    def w1c(t):
        return slice(2 + t * 2 * KO, 2 + t * 2 * KO + KO)

    def w2c(t):
        return slice(2 + t * 2 * KO + KO, 2 + (t + 1) * 2 * KO)

    dma_sem = nc.alloc_semaphore("out_dma_sem")
    dma_sem2 = nc.alloc_semaphore("out_dma_sem2")

    with tile.TileContext(nc) as tc:
        with (
            tc.tile_pool(name="io", bufs=1) as io,
            tc.tile_pool(name="work", bufs=1) as work,
            tc.tile_pool(name="pp", bufs=1, space="PSUM") as pp,
        ):
            wsb = io.tile([128, WCOLS], bf16, tag="wsb")
            xsb = io.tile([128, NT, BS], bf16, tag="xsb")

            # inputs: [b2|w1t0|w2t0] from SP; [w1t1|w2t1|idxs] and x halves
            # from ACT so HWDGE order is wt0, wt1, x0, x1
            nc.sync.dma_start(out=wsb[:, 0:W0E], in_=wb[:, 0:W0E])
            nc.scalar.dma_start(out=wsb[:, W0E:WCOLS], in_=wb[:, W0E:WCOLS])
            for c in range(NCH):
                nc.scalar.dma_start(out=xsb[:, :, c * CB:(c + 1) * CB],
                                    in_=xb[:, :, c * CB:(c + 1) * CB])

            b2ap = wsb[:, 0:2].bitcast(f32)
            idxs_ap = wsb[:, WCOLS - NIDX:WCOLS].bitcast(i16)

            # |w1| per tile on ACT
            a4 = []
            for t in range(NT):
                a = work.tile([128, KO], bf16, tag=f"a4{t}")
                nc.scalar.activation(a, wsb[:, w1c(t)], AF.Abs)
                a4.append(a)

            # S chains on DVE
            s_t, t_t = [], []
            for t in range(NT):
                s4 = work.tile([128, KO], bf16, tag=f"s4{t}")
                nc.vector.tensor_mul(s4, wsb[:, w1c(t)], wsb[:, w2c(t)])
                s2 = work.tile([128, KO // 2], bf16, tag=f"s2{t}")
                nc.vector.tensor_add(s2, s4[:, 0:KO // 2], s4[:, KO // 2:KO])
                st = work.tile([128, OS], bf16, tag=f"st{t}")
                nc.vector.tensor_add(st, s2[:, 0:OS], s2[:, OS:2 * OS])
                s_t.append(st)

            # T tile0 product on GpSimd (parallel with DVE's chains)
            t4_0 = work.tile([128, KO], bf16, tag="t4_0")
            nc.gpsimd.tensor_mul(t4_0, a4[0], wsb[:, w2c(0)])
            # T tile1 product + both tiles' adds on DVE
            t4_1 = work.tile([128, KO], bf16, tag="t4_1")
            nc.vector.tensor_mul(t4_1, a4[1], wsb[:, w2c(1)])
            t2_0 = work.tile([128, KO // 2], bf16, tag="t2_0")
            nc.vector.tensor_add(t2_0, t4_0[:, 0:KO // 2], t4_0[:, KO // 2:KO])
            tt_0 = work.tile([128, OS], bf16, tag="tt_0")
            nc.vector.tensor_add(tt_0, t2_0[:, 0:OS], t2_0[:, OS:2 * OS])
            t_t.append(tt_0)
            t2_1 = work.tile([128, KO // 2], bf16, tag="t2_1")
            nc.vector.tensor_add(t2_1, t4_1[:, 0:KO // 2], t4_1[:, KO // 2:KO])
            tt_1 = work.tile([128, OS], bf16, tag="tt_1")
            nc.vector.tensor_add(tt_1, t2_1[:, 0:OS], t2_1[:, OS:2 * OS])
            t_t.append(tt_1)

            # |x| per chunk on ACT (chunk 0 available before chunk 1 lands)
            xa = work.tile([128, NT, BS], bf16, tag="xa")
            for c in range(NCH):
                nc.scalar.activation(xa[:, :, c * CB:(c + 1) * CB],
                                     xsb[:, :, c * CB:(c + 1) * CB], AF.Abs)

            # matmuls: S parts of both chunks first, T parts after
            psums = []
            for c in range(NCH):
                psum = pp.tile([128, CB], f32, tag=f"ps{c}")
                psums.append(psum)
            for c in range(NCH):
                sl = slice(c * CB, (c + 1) * CB)
                nc.tensor.matmul(psums[c], lhsT=s_t[0], rhs=xsb[:, 0, sl],
                                 start=True, stop=False)
                nc.tensor.matmul(psums[c], lhsT=s_t[1], rhs=xsb[:, 1, sl],
                                 start=False, stop=False)
            for c in range(NCH - 1, -1, -1):
                sl = slice(c * CB, (c + 1) * CB)
                nc.tensor.matmul(psums[c], lhsT=t_t[0], rhs=xa[:, 0, sl],
                                 start=False, stop=False)
            for c in range(NCH):
                sl = slice(c * CB, (c + 1) * CB)
                nc.tensor.matmul(psums[c], lhsT=t_t[1], rhs=xa[:, 1, sl],
                                 start=False, stop=True)

            s2c = float(SCALE) / 2.0
            osb = work.tile([128, 1, BS], bf16, tag="osb")
            # chunk 0 epilogue on ACT, chunk 1 on DVE (parallel)
            nc.scalar.activation(osb[:, 0, 0:CB], psums[0], AF.Identity,
                                 bias=b2ap, scale=s2c)
            nc.vector.tensor_scalar(osb[:, 0, CB:BS], psums[1], s2c, b2ap,
                                    op0=OP.mult, op1=OP.add)

            # output: per-chunk prepared SWDGE scatters (identity idxs),
            # each fired by its own trigger as its epilogue completes
            nc.gpsimd.dma_scatter_add(outb[:, 0:CB], osb[:, :, 0:CB],
                                      idxs_ap, 128, 128, CB,
                                      elem_step=BS,
                                      prepare_only=True, sem=dma_sem)
            nc.gpsimd.trigger_dma(count=None)
            nc.gpsimd.dma_scatter_add(outb[:, CB:BS], osb[:, :, CB:BS],
                                      idxs_ap, 128, 128, CB,
                                      elem_step=BS,
                                      prepare_only=True, sem=dma_sem2)
            nc.gpsimd.trigger_dma(count=None)

    nc.compile()

    # Tile assigns the prepared scatter a DMASW lane and makes the block
    # exit wait on that lane semaphore, but routes the user sem= into
    # on_update[0] instead of the lane sem — nothing ever bumps the lane.
    # Point the prep's DMA-completion update at the lane semaphore (the
    # same attachment a normal Pool DMA gets), matching the exit wait.
    fn = nc.m.functions[0]
    preps = []
    lanes = {}
    for blk in fn.blocks:
        for inst in blk.instructions:
            if type(inst).__name__ == "InstDMAScatterAddAnt":
                preps.append(inst)
            si = inst.sync_info
            if si is not None:
                for w in (si.on_wait or []):
                    nm = getattr(w, "ant_name", None)
                    if nm and nm.startswith("DMASW") and w.wait_value == 16:
                        lanes[nm] = w.id
    assert preps and len(lanes) == len(preps), (len(preps), lanes)
    for i, prep in enumerate(preps):
        nm = f"DMASW{i}_" + next(iter(lanes)).split("_")[1]
        u0 = prep.sync_info.on_update[0]
        u0.id = lanes[nm]
        u0.ant_name = nm

    _CACHE["nc"] = nc
    return nc


def _kan_numpy(x, w1, b1, w2, b2):
    # exact fallback, chunked over batch to bound memory
    out = np.empty((x.shape[0], w1.shape[0]), dtype=np.float32)
    d = (w1.shape[0] + w1.shape[1]) / 2
    s = 1.0 / np.sqrt(d * w1.shape[2])
    for lo in range(0, x.shape[0], 128):
        hi = min(lo + 128, x.shape[0])
        h = x[lo:hi, None, :, None] * w1[None] + b1[None]
        np.maximum(h, 0.0, out=h)
        out[lo:hi] = np.einsum("boik,oik->bo", h, w2) * s
    return out + b2[None, :]


def kernel(x, w1, b1, w2, b2):
    x = np.ascontiguousarray(x, dtype=np.float32)
    w1 = np.asarray(w1, dtype=np.float32)
    b1 = np.asarray(b1, dtype=np.float32)
    w2 = np.asarray(w2, dtype=np.float32)
    b2 = np.asarray(b2, dtype=np.float32)

    if x.shape != (B, DIN) or w1.shape != (DOUT, DIN, K) or np.any(b1):
        return _kan_numpy(x, w1, b1, w2, b2)

    from concourse.bass_utils import run_bass_kernel_spmd
    from ml_dtypes import bfloat16

    nc = _build_nc()

    xT = np.ascontiguousarray(x.T).astype(bfloat16)     # (DIN, B)
    w1T = w1.transpose(1, 2, 0).astype(bfloat16)        # (DIN, K, DOUT)
    w2T = w2.transpose(1, 2, 0).astype(bfloat16)
    # token i reads its destination from idxs[i % 16, i // 16]
    idxs = np.tile(
        np.ascontiguousarray(np.arange(128, dtype=np.int16).reshape(NIDX, 16).T),
        (8, 1))

    in_maps = []
    for core in range(N_CORES):
        bi, oj = divmod(core, OG)
        wcore = np.empty((128, WCOLS), dtype=bfloat16)
        b2f = np.ascontiguousarray(
            b2[oj * OS:(oj + 1) * OS]).reshape(OS, 1)
        wcore[:, 0:2] = b2f.view(np.uint16).view(bfloat16)
        for t in range(NT):
            r = slice(t * 128, (t + 1) * 128)
            o = slice(oj * OS, (oj + 1) * OS)
            wcore[:, 2 + t * 2 * KO:2 + t * 2 * KO + KO] = \
                w1T[r, :, o].reshape(128, KO)
            wcore[:, 2 + t * 2 * KO + KO:2 + (t + 1) * 2 * KO] = \
                w2T[r, :, o].reshape(128, KO)
        wcore[:, WCOLS - NIDX:WCOLS] = idxs.view(bfloat16)
        xcore = np.ascontiguousarray(
            xT[:, bi * BS:(bi + 1) * BS].reshape(NT, 128, BS)
            .transpose(1, 0, 2))
        in_maps.append({"wb": wcore, "xb": xcore})

    res = run_bass_kernel_spmd(nc, in_maps, core_ids=list(range(N_CORES)))

    out = np.empty((B, DOUT), dtype=np.float32)
    for core in range(N_CORES):
        bi, oj = divmod(core, OG)
        out[bi * BS:(bi + 1) * BS, oj * OS:(oj + 1) * OS] = \
            res.results[core]["outb"].T.astype(np.float32)
    return out


# revision 18
# speedup vs baseline: 1.0174x; 1.0174x over previous
"""KAN layer kernel for TRN2, 8-core SPMD.

Math: out[b,o] = sum_{i,k} relu(x[b,i]*w1[o,i,k] + b1[o,i,k]) * w2[o,i,k] / 32 + b2[o]
With b1 == 0 (guaranteed by the generator) the relu factorizes via
relu(z) = (z + |z|)/2 and |x*w1| = |x|*|w1|:
    S[i,o] = sum_k w1*w2        T[i,o] = sum_k |w1|*w2
    out = (x @ S + |x| @ T) * (1/64) + b2
Two bf16 matmuls plus elementwise prep spread across DVE (products,
k-sums), ACT (|.|, epilogue), GpSimd (one product leg). The output store
is a pair of per-chunk prepared SWDGE scatters (identity indices), each
fired by trigger_dma as its epilogue completes — this skips the HWDGE +
DGE-delay latency of a normal DMA; the scatters ADD into the
zero-initialized output buffer, which equals a store.

Sharding: 4 batch groups x 2 dout groups (core = bi*2 + oj).
Wire format is bf16 (host casts; all arithmetic on device; f32 psum
accumulation); output returns in bf16 and is upcast on host.
"""

import numpy as np

B, DIN, DOUT, K = 2048, 256, 256, 4
N_CORES = 8
BG, OG = 4, 2                      # batch groups x dout groups
BS, OS = B // BG, DOUT // OG       # 512 batch rows, 128 dout cols per core
SCALE = 1.0 / np.sqrt(((DOUT + DIN) / 2) * K)   # 1/32
NT = DIN // 128                    # i-tiles (2)
KO = K * OS                        # 512 cols per weight tensor slab
NIDX = 8                           # identity scatter idxs: 8 int16 cols
WCOLS = 2 + NT * 2 * KO + NIDX     # b2 bits + w1/w2 slabs + idxs
NCH = 2                            # batch chunks
CB = BS // NCH                     # 256 batch cols per chunk

_CACHE = {}


def _build_nc():
    if "nc" in _CACHE:
        return _CACHE["nc"]
    import concourse.bacc as bacc
    import concourse.tile as tile
    from concourse import mybir

    f32 = mybir.dt.float32
    bf16 = mybir.dt.bfloat16
    i16 = mybir.dt.int16
    AF = mybir.ActivationFunctionType
    OP = mybir.AluOpType

    nc = bacc.Bacc("TRN2", target_bir_lowering=False, debug=False,
                   num_devices=N_CORES)
    wb = nc.dram_tensor("wb", [128, WCOLS], bf16, kind="ExternalInput")
    xb = nc.dram_tensor("xb", [128, NT, BS], bf16, kind="ExternalInput")
    outb = nc.dram_tensor("outb", [128, BS], bf16, kind="ExternalOutput")

    W0E = 2 + 2 * KO               # end of [b2 | w1t0 | w2t0]

    def w1c(t):
        return slice(2 + t * 2 * KO, 2 + t * 2 * KO + KO)

    def w2c(t):
        return slice(2 + t * 2 * KO + KO, 2 + (t + 1) * 2 * KO)

    dma_sem = nc.alloc_semaphore("out_dma_sem")
    dma_sem2 = nc.alloc_semaphore("out_dma_sem2")

    with tile.TileContext(nc) as tc:
        with (
            tc.tile_pool(name="io", bufs=1) as io,
            tc.tile_pool(name="work", bufs=1) as work,
            tc.tile_pool(name="pp", bufs=1, space="PSUM") as pp,
        ):
            wsb = io.tile([128, WCOLS], bf16, tag="wsb")
            xsb = io.tile([128, NT, BS], bf16, tag="xsb")

            # inputs: [b2|w1t0|w2t0] from SP; [w1t1|w2t1|idxs] and x halves
            # from ACT so HWDGE order is wt0, wt1, x0, x1
            nc.sync.dma_start(out=wsb[:, 0:W0E], in_=wb[:, 0:W0E])
            nc.scalar.dma_start(out=wsb[:, W0E:WCOLS], in_=wb[:, W0E:WCOLS])
            for c in range(NCH):
                nc.scalar.dma_start(out=xsb[:, :, c * CB:(c + 1) * CB],
                                    in_=xb[:, :, c * CB:(c + 1) * CB])

            b2ap = wsb[:, 0:2].bitcast(f32)
            idxs_ap = wsb[:, WCOLS - NIDX:WCOLS].bitcast(i16)

            # |w1| per tile on ACT
            a4 = []
            for t in range(NT):
                a = work.tile([128, KO], bf16, tag=f"a4{t}")
                nc.scalar.activation(a, wsb[:, w1c(t)], AF.Abs)
                a4.append(a)

            # S chains on DVE
            s_t, t_t = [], []
            for t in range(NT):
                s4 = work.tile([128, KO], bf16, tag=f"s4{t}")
                nc.vector.tensor_mul(s4, wsb[:, w1c(t)], wsb[:, w2c(t)])
                s2 = work.tile([128, KO // 2], bf16, tag=f"s2{t}")
                nc.vector.tensor_add(s2, s4[:, 0:KO // 2], s4[:, KO // 2:KO])
                st = work.tile([128, OS], bf16, tag=f"st{t}")
                nc.vector.tensor_add(st, s2[:, 0:OS], s2[:, OS:2 * OS])
                s_t.append(st)

            # T tile0 product on GpSimd (parallel with DVE's chains)
            t4_0 = work.tile([128, KO], bf16, tag="t4_0")
            nc.gpsimd.tensor_mul(t4_0, a4[0], wsb[:, w2c(0)])
            # T tile1 product + both tiles' adds on DVE
            t4_1 = work.tile([128, KO], bf16, tag="t4_1")
            nc.vector.tensor_mul(t4_1, a4[1], wsb[:, w2c(1)])
            t2_0 = work.tile([128, KO // 2], bf16, tag="t2_0")
            nc.vector.tensor_add(t2_0, t4_0[:, 0:KO // 2], t4_0[:, KO // 2:KO])
            tt_0 = work.tile([128, OS], bf16, tag="tt_0")
            nc.vector.tensor_add(tt_0, t2_0[:, 0:OS], t2_0[:, OS:2 * OS])
            t_t.append(tt_0)
            t2_1 = work.tile([128, KO // 2], bf16, tag="t2_1")
            nc.vector.tensor_add(t2_1, t4_1[:, 0:KO // 2], t4_1[:, KO // 2:KO])
            tt_1 = work.tile([128, OS], bf16, tag="tt_1")
            nc.vector.tensor_add(tt_1, t2_1[:, 0:OS], t2_1[:, OS:2 * OS])
            t_t.append(tt_1)

            # |x| per chunk on ACT; chunk 1 split per tile so the tile-0
            # half (feeding the T0 matmul) is ready sooner
            xa = work.tile([128, NT, BS], bf16, tag="xa")
            nc.scalar.activation(xa[:, :, 0:CB], xsb[:, :, 0:CB], AF.Abs)
            nc.scalar.activation(xa[:, 0, CB:BS], xsb[:, 0, CB:BS], AF.Abs)
            nc.scalar.activation(xa[:, 1, CB:BS], xsb[:, 1, CB:BS], AF.Abs)

            # matmuls: S parts of both chunks first, T parts after
            psums = []
            for c in range(NCH):
                psum = pp.tile([128, CB], f32, tag=f"ps{c}")
                psums.append(psum)
            for c in range(NCH):
                sl = slice(c * CB, (c + 1) * CB)
                nc.tensor.matmul(psums[c], lhsT=s_t[0], rhs=xsb[:, 0, sl],
                                 start=True, stop=False)
                nc.tensor.matmul(psums[c], lhsT=s_t[1], rhs=xsb[:, 1, sl],
                                 start=False, stop=False)
            for c in range(NCH - 1, -1, -1):
                sl = slice(c * CB, (c + 1) * CB)
                nc.tensor.matmul(psums[c], lhsT=t_t[0], rhs=xa[:, 0, sl],
                                 start=False, stop=False)
            for c in range(NCH):
                sl = slice(c * CB, (c + 1) * CB)
                nc.tensor.matmul(psums[c], lhsT=t_t[1], rhs=xa[:, 1, sl],
                                 start=False, stop=True)

            s2c = float(SCALE) / 2.0
            osb = work.tile([128, 1, BS], bf16, tag="osb")
            # chunk 0 epilogue on ACT, chunk 1 on DVE (parallel)
            nc.scalar.activation(osb[:, 0, 0:CB], psums[0], AF.Identity,
                                 bias=b2ap, scale=s2c)
            nc.vector.tensor_scalar(osb[:, 0, CB:BS], psums[1], s2c, b2ap,
                                    op0=OP.mult, op1=OP.add)

            # output: per-chunk prepared SWDGE scatters (identity idxs),
            # each fired by its own trigger as its epilogue completes
            nc.gpsimd.dma_scatter_add(outb[:, 0:CB], osb[:, :, 0:CB],
                                      idxs_ap, 128, 128, CB,
                                      elem_step=BS,
                                      prepare_only=True, sem=dma_sem)
            nc.gpsimd.trigger_dma(count=None)
            nc.gpsimd.dma_scatter_add(outb[:, CB:BS], osb[:, :, CB:BS],
                                      idxs_ap, 128, 128, CB,
                                      elem_step=BS,
                                      prepare_only=True, sem=dma_sem2)
            nc.gpsimd.trigger_dma(count=None)

    nc.compile()

    # Tile assigns the prepared scatter a DMASW lane and makes the block
    # exit wait on that lane semaphore, but routes the user sem= into
    # on_update[0] instead of the lane sem — nothing ever bumps the lane.
    # Point the prep's DMA-completion update at the lane semaphore (the
    # same attachment a normal Pool DMA gets), matching the exit wait.
    fn = nc.m.functions[0]
    preps = []
    lanes = {}
    for blk in fn.blocks:
        for inst in blk.instructions:
            if type(inst).__name__ == "InstDMAScatterAddAnt":
                preps.append(inst)
            si = inst.sync_info
            if si is not None:
                for w in (si.on_wait or []):
                    nm = getattr(w, "ant_name", None)
                    if nm and nm.startswith("DMASW") and w.wait_value == 16:
                        lanes[nm] = w.id
    assert preps and len(lanes) == len(preps), (len(preps), lanes)
    for i, prep in enumerate(preps):
        nm = f"DMASW{i}_" + next(iter(lanes)).split("_")[1]
        u0 = prep.sync_info.on_update[0]
        u0.id = lanes[nm]
        u0.ant_name = nm

    _CACHE["nc"] = nc
    return nc


def _kan_numpy(x, w1, b1, w2, b2):
    # exact fallback, chunked over batch to bound memory
    out = np.empty((x.shape[0], w1.shape[0]), dtype=np.float32)
    d = (w1.shape[0] + w1.shape[1]) / 2
    s = 1.0 / np.sqrt(d * w1.shape[2])
    for lo in range(0, x.shape[0], 128):
        hi = min(lo + 128, x.shape[0])
        h = x[lo:hi, None, :, None] * w1[None] + b1[None]
        np.maximum(h, 0.0, out=h)
        out[lo:hi] = np.einsum("boik,oik->bo", h, w2) * s
    return out + b2[None, :]


def kernel(x, w1, b1, w2, b2):
    x = np.ascontiguousarray(x, dtype=np.float32)
    w1 = np.asarray(w1, dtype=np.float32)
    b1 = np.asarray(b1, dtype=np.float32)
    w2 = np.asarray(w2, dtype=np.float32)
    b2 = np.asarray(b2, dtype=np.float32)

    if x.shape != (B, DIN) or w1.shape != (DOUT, DIN, K) or np.any(b1):
        return _kan_numpy(x, w1, b1, w2, b2)

    from concourse.bass_utils import run_bass_kernel_spmd
    from ml_dtypes import bfloat16

    nc = _build_nc()

    xT = np.ascontiguousarray(x.T).astype(bfloat16)     # (DIN, B)
    w1T = w1.transpose(1, 2, 0).astype(bfloat16)        # (DIN, K, DOUT)
    w2T = w2.transpose(1, 2, 0).astype(bfloat16)
    # token i reads its destination from idxs[i % 16, i // 16]
    idxs = np.tile(
        np.ascontiguousarray(np.arange(128, dtype=np.int16).reshape(NIDX, 16).T),
        (8, 1))

    in_maps = []
    for core in range(N_CORES):
        bi, oj = divmod(core, OG)
        wcore = np.empty((128, WCOLS), dtype=bfloat16)
        b2f = np.ascontiguousarray(
            b2[oj * OS:(oj + 1) * OS]).reshape(OS, 1)
        wcore[:, 0:2] = b2f.view(np.uint16).view(bfloat16)
        for t in range(NT):
            r = slice(t * 128, (t + 1) * 128)
            o = slice(oj * OS, (oj + 1) * OS)
            wcore[:, 2 + t * 2 * KO:2 + t * 2 * KO + KO] = \
                w1T[r, :, o].reshape(128, KO)
            wcore[:, 2 + t * 2 * KO + KO:2 + (t + 1) * 2 * KO] = \
                w2T[r, :, o].reshape(128, KO)
        wcore[:, WCOLS - NIDX:WCOLS] = idxs.view(bfloat16)
        xcore = np.ascontiguousarray(
            xT[:, bi * BS:(bi + 1) * BS].reshape(NT, 128, BS)
            .transpose(1, 0, 2))
        in_maps.append({"wb": wcore, "xb": xcore})

    res = run_bass_kernel_spmd(nc, in_maps, core_ids=list(range(N_CORES)))

    out = np.empty((B, DOUT), dtype=np.float32)
    for core in range(N_CORES):
        bi, oj = divmod(core, OG)
        out[bi * BS:(bi + 1) * BS, oj * OS:(oj + 1) * OS] = \
            res.results[core]["outb"].T.astype(np.float32)
    return out


# revision 19
# speedup vs baseline: 1.0209x; 1.0035x over previous
"""KAN layer kernel for TRN2, 8-core SPMD.

Math: out[b,o] = sum_{i,k} relu(x[b,i]*w1[o,i,k] + b1[o,i,k]) * w2[o,i,k] / 32 + b2[o]
With b1 == 0 (guaranteed by the generator) the relu factorizes via
relu(z) = (z + |z|)/2 and |x*w1| = |x|*|w1|:
    S[i,o] = sum_k w1*w2        T[i,o] = sum_k |w1|*w2
    out = (x @ S + |x| @ T) * (1/64) + b2
Two bf16 matmuls plus elementwise prep spread across DVE (products,
k-sums), ACT (|.|, epilogue), GpSimd (one product leg). The output store
is a pair of per-chunk prepared SWDGE scatters (identity indices), each
fired by trigger_dma as its epilogue completes — this skips the HWDGE +
DGE-delay latency of a normal DMA; the scatters ADD into the
zero-initialized output buffer, which equals a store.

Sharding: 4 batch groups x 2 dout groups (core = bi*2 + oj).
Wire format is bf16 (host casts; all arithmetic on device; f32 psum
accumulation); output returns in bf16 and is upcast on host.
"""

import numpy as np

B, DIN, DOUT, K = 2048, 256, 256, 4
N_CORES = 8
BG, OG = 4, 2                      # batch groups x dout groups
BS, OS = B // BG, DOUT // OG       # 512 batch rows, 128 dout cols per core
SCALE = 1.0 / np.sqrt(((DOUT + DIN) / 2) * K)   # 1/32
NT = DIN // 128                    # i-tiles (2)
KO = K * OS                        # 512 cols per weight tensor slab
NIDX = 8                           # identity scatter idxs: 8 int16 cols
WCOLS = 2 + NT * 2 * KO + NIDX     # b2 bits + w1/w2 slabs + idxs
NCH = 2                            # batch chunks
CB = BS // NCH                     # 256 batch cols per chunk

_CACHE = {}


def _build_nc():
    if "nc" in _CACHE:
        return _CACHE["nc"]
    import concourse.bacc as bacc
    import concourse.tile as tile
    from concourse import mybir

    f32 = mybir.dt.float32
    bf16 = mybir.dt.bfloat16
    i16 = mybir.dt.int16
    AF = mybir.ActivationFunctionType
    OP = mybir.AluOpType

    nc = bacc.Bacc("TRN2", target_bir_lowering=False, debug=False,
                   num_devices=N_CORES)
    wb = nc.dram_tensor("wb", [128, WCOLS], bf16, kind="ExternalInput")
    xb = nc.dram_tensor("xb", [128, NT, BS], bf16, kind="ExternalInput")
    outb = nc.dram_tensor("outb", [128, BS], bf16, kind="ExternalOutput")

    W0E = 2 + 2 * KO               # end of [b2 | w1t0 | w2t0]

    def w1c(t):
        return slice(2 + t * 2 * KO, 2 + t * 2 * KO + KO)

    def w2c(t):
        return slice(2 + t * 2 * KO + KO, 2 + (t + 1) * 2 * KO)

    dma_sem = nc.alloc_semaphore("out_dma_sem")
    dma_sem2 = nc.alloc_semaphore("out_dma_sem2")

    with tile.TileContext(nc) as tc:
        with (
            tc.tile_pool(name="io", bufs=1) as io,
            tc.tile_pool(name="work", bufs=1) as work,
            tc.tile_pool(name="pp", bufs=1, space="PSUM") as pp,
        ):
            wsb = io.tile([128, WCOLS], bf16, tag="wsb")
            xsb = io.tile([128, NT, BS], bf16, tag="xsb")

            # inputs: [b2|w1t0|w2t0] from SP; [w1t1|w2t1|idxs] and x halves
            # from ACT so HWDGE order is wt0, wt1, x0, x1
            nc.sync.dma_start(out=wsb[:, 0:W0E], in_=wb[:, 0:W0E])
            nc.scalar.dma_start(out=wsb[:, W0E:WCOLS], in_=wb[:, W0E:WCOLS])
            for c in range(NCH):
                nc.scalar.dma_start(out=xsb[:, :, c * CB:(c + 1) * CB],
                                    in_=xb[:, :, c * CB:(c + 1) * CB])

            b2ap = wsb[:, 0:2].bitcast(f32)
            idxs_ap = wsb[:, WCOLS - NIDX:WCOLS].bitcast(i16)

            # |w1| per tile on ACT
            a4 = []
            for t in range(NT):
                a = work.tile([128, KO], bf16, tag=f"a4{t}")
                nc.scalar.activation(a, wsb[:, w1c(t)], AF.Abs)
                a4.append(a)

            # S chains on DVE
            s_t, t_t = [], []
            for t in range(NT):
                s4 = work.tile([128, KO], bf16, tag=f"s4{t}")
                nc.vector.tensor_mul(s4, wsb[:, w1c(t)], wsb[:, w2c(t)])
                s2 = work.tile([128, KO // 2], bf16, tag=f"s2{t}")
                nc.vector.tensor_add(s2, s4[:, 0:KO // 2], s4[:, KO // 2:KO])
                st = work.tile([128, OS], bf16, tag=f"st{t}")
                nc.vector.tensor_add(st, s2[:, 0:OS], s2[:, OS:2 * OS])
                s_t.append(st)

            # T tile0 product on GpSimd (parallel with DVE's chains)
            t4_0 = work.tile([128, KO], bf16, tag="t4_0")
            nc.gpsimd.tensor_mul(t4_0, a4[0], wsb[:, w2c(0)])
            # T tile1 product + both tiles' adds on DVE
            t4_1 = work.tile([128, KO], bf16, tag="t4_1")
            nc.vector.tensor_mul(t4_1, a4[1], wsb[:, w2c(1)])
            t2_0 = work.tile([128, KO // 2], bf16, tag="t2_0")
            nc.vector.tensor_add(t2_0, t4_0[:, 0:KO // 2], t4_0[:, KO // 2:KO])
            tt_0 = work.tile([128, OS], bf16, tag="tt_0")
            nc.vector.tensor_add(tt_0, t2_0[:, 0:OS], t2_0[:, OS:2 * OS])
            t_t.append(tt_0)
            t2_1 = work.tile([128, KO // 2], bf16, tag="t2_1")
            nc.vector.tensor_add(t2_1, t4_1[:, 0:KO // 2], t4_1[:, KO // 2:KO])
            tt_1 = work.tile([128, OS], bf16, tag="tt_1")
            nc.vector.tensor_add(tt_1, t2_1[:, 0:OS], t2_1[:, OS:2 * OS])
            t_t.append(tt_1)

            # |x| per chunk on ACT; chunk 1 split per tile so the tile-0
            # half (feeding the T0 matmul) is ready sooner
            xa = work.tile([128, NT, BS], bf16, tag="xa")
            nc.scalar.activation(xa[:, :, 0:CB], xsb[:, :, 0:CB], AF.Abs)
            nc.scalar.activation(xa[:, 0, CB:BS], xsb[:, 0, CB:BS], AF.Abs)
            nc.scalar.activation(xa[:, 1, CB:BS], xsb[:, 1, CB:BS], AF.Abs)

            # matmuls over 4 narrow psum groups (closes stagger so the four
            # small epilogues pipeline round-robin across ACT and DVE)
            NQ, QB = 4, BS // 4
            psums = []
            for c in range(NQ):
                psum = pp.tile([128, QB], f32, tag=f"ps{c}")
                psums.append(psum)
            for c in range(NQ):
                sl = slice(c * QB, (c + 1) * QB)
                nc.tensor.matmul(psums[c], lhsT=s_t[0], rhs=xsb[:, 0, sl],
                                 start=True, stop=False)
                nc.tensor.matmul(psums[c], lhsT=s_t[1], rhs=xsb[:, 1, sl],
                                 start=False, stop=False)
            for c in range(NQ):
                sl = slice(c * QB, (c + 1) * QB)
                nc.tensor.matmul(psums[c], lhsT=t_t[0], rhs=xa[:, 0, sl],
                                 start=False, stop=False)
            for c in range(NQ):
                sl = slice(c * QB, (c + 1) * QB)
                nc.tensor.matmul(psums[c], lhsT=t_t[1], rhs=xa[:, 1, sl],
                                 start=False, stop=True)

            s2c = float(SCALE) / 2.0
            osb = work.tile([128, 1, BS], bf16, tag="osb")
            for c in range(NQ):
                sl = slice(c * QB, (c + 1) * QB)
                if c % 2 == 0:
                    nc.scalar.activation(osb[:, 0, sl], psums[c], AF.Identity,
                                         bias=b2ap, scale=s2c)
                else:
                    nc.vector.tensor_scalar(osb[:, 0, sl], psums[c], s2c,
                                            b2ap, op0=OP.mult, op1=OP.add)

            # output: per-chunk prepared SWDGE scatters (identity idxs),
            # each fired by its own trigger as its epilogue completes
            nc.gpsimd.dma_scatter_add(outb[:, 0:CB], osb[:, :, 0:CB],
                                      idxs_ap, 128, 128, CB,
                                      elem_step=BS,
                                      prepare_only=True, sem=dma_sem)
            nc.gpsimd.trigger_dma(count=None)
            nc.gpsimd.dma_scatter_add(outb[:, CB:BS], osb[:, :, CB:BS],
                                      idxs_ap, 128, 128, CB,
                                      elem_step=BS,
                                      prepare_only=True, sem=dma_sem2)
            nc.gpsimd.trigger_dma(count=None)

    nc.compile()

    # Tile assigns the prepared scatter a DMASW lane and makes the block
    # exit wait on that lane semaphore, but routes the user sem= into
    # on_update[0] instead of the lane sem — nothing ever bumps the lane.
    # Point the prep's DMA-completion update at the lane semaphore (the
    # same attachment a normal Pool DMA gets), matching the exit wait.
    fn = nc.m.functions[0]
    preps = []
    lanes = {}
    for blk in fn.blocks:
        for inst in blk.instructions:
            if type(inst).__name__ == "InstDMAScatterAddAnt":
                preps.append(inst)
            si = inst.sync_info
            if si is not None:
                for w in (si.on_wait or []):
                    nm = getattr(w, "ant_name", None)
                    if nm and nm.startswith("DMASW") and w.wait_value == 16:
                        lanes[nm] = w.id
    assert preps and len(lanes) == len(preps), (len(preps), lanes)
    for i, prep in enumerate(preps):
        nm = f"DMASW{i}_" + next(iter(lanes)).split("_")[1]
        u0 = prep.sync_info.on_update[0]
        u0.id = lanes[nm]
        u0.ant_name = nm

    _CACHE["nc"] = nc
    return nc


def _kan_numpy(x, w1, b1, w2, b2):
    # exact fallback, chunked over batch to bound memory
    out = np.empty((x.shape[0], w1.shape[0]), dtype=np.float32)
    d = (w1.shape[0] + w1.shape[1]) / 2
    s = 1.0 / np.sqrt(d * w1.shape[2])
    for lo in range(0, x.shape[0], 128):
        hi = min(lo + 128, x.shape[0])
        h = x[lo:hi, None, :, None] * w1[None] + b1[None]
        np.maximum(h, 0.0, out=h)
        out[lo:hi] = np.einsum("boik,oik->bo", h, w2) * s
    return out + b2[None, :]


def kernel(x, w1, b1, w2, b2):
    x = np.ascontiguousarray(x, dtype=np.float32)
    w1 = np.asarray(w1, dtype=np.float32)
    b1 = np.asarray(b1, dtype=np.float32)
    w2 = np.asarray(w2, dtype=np.float32)
    b2 = np.asarray(b2, dtype=np.float32)

    if x.shape != (B, DIN) or w1.shape != (DOUT, DIN, K) or np.any(b1):
        return _kan_numpy(x, w1, b1, w2, b2)

    from concourse.bass_utils import run_bass_kernel_spmd
    from ml_dtypes import bfloat16

    nc = _build_nc()

    xT = np.ascontiguousarray(x.T).astype(bfloat16)     # (DIN, B)
    w1T = w1.transpose(1, 2, 0).astype(bfloat16)        # (DIN, K, DOUT)
    w2T = w2.transpose(1, 2, 0).astype(bfloat16)
    # token i reads its destination from idxs[i % 16, i // 16]
    idxs = np.tile(
        np.ascontiguousarray(np.arange(128, dtype=np.int16).reshape(NIDX, 16).T),
        (8, 1))

    in_maps = []
    for core in range(N_CORES):
        bi, oj = divmod(core, OG)
        wcore = np.empty((128, WCOLS), dtype=bfloat16)
        b2f = np.ascontiguousarray(
            b2[oj * OS:(oj + 1) * OS]).reshape(OS, 1)
        wcore[:, 0:2] = b2f.view(np.uint16).view(bfloat16)
        for t in range(NT):
            r = slice(t * 128, (t + 1) * 128)
            o = slice(oj * OS, (oj + 1) * OS)
            wcore[:, 2 + t * 2 * KO:2 + t * 2 * KO + KO] = \
                w1T[r, :, o].reshape(128, KO)
            wcore[:, 2 + t * 2 * KO + KO:2 + (t + 1) * 2 * KO] = \
                w2T[r, :, o].reshape(128, KO)
        wcore[:, WCOLS - NIDX:WCOLS] = idxs.view(bfloat16)
        xcore = np.ascontiguousarray(
            xT[:, bi * BS:(bi + 1) * BS].reshape(NT, 128, BS)
            .transpose(1, 0, 2))
        in_maps.append({"wb": wcore, "xb": xcore})

    res = run_bass_kernel_spmd(nc, in_maps, core_ids=list(range(N_CORES)))

    out = np.empty((B, DOUT), dtype=np.float32)
    for core in range(N_CORES):
        bi, oj = divmod(core, OG)
        out[bi * BS:(bi + 1) * BS, oj * OS:(oj + 1) * OS] = \
            res.results[core]["outb"].T.astype(np.float32)
    return out
